# revision 19
# baseline (speedup 1.0000x reference)
"""CoattentionNet Trainium2 kernel (fp8 DoubleRow version).

Reference computation (per batch b, E = emb[tokens_b] in [L=256, D=256]):
    C   = tanh(E @ W_b @ E^T)                  [L, L]
    a   = softmax_l(max_m C[l, m])             [L]
    f_w = sum_l a[l] * E[l, :]                 [D]
    out = f_w @ lin_w^T + lin_b                [O=1000]

Math used on device:
  * tanh is monotonic -> rowmax(tanh(M)) = tanh(rowmax(M)); tanh in [-1,1] so
    softmax needs no max-subtraction.
  * softmax normalization commutes with the weighted sum and the final linear:
    unnormalized w = exp(tanh(rowmax)) feeds the weighted-sum matmuls, and
    F^T is scaled by 1/Z right before the output linear.
  * The C = tanh(M) path only steers a softmax that is nearly uniform (|M| is
    tiny), so H and M tolerate fp8: host ships E^T pre-scaled by 16 in
    fp8e4m3, W_b scaled by 16 in fp8; the PSUM results carry exact
    power-of-two scales undone in the ACT cast (x1/16) and the tanh
    activation (scale=1/256). The weighted sum itself uses bf16 E.

Per batch on PE:
    H  = W_b @ E^T   fp8 DoubleRow (K=256 in one instr)   [d, m]
    M  = E @ H       fp8 DoubleRow, lhsT = E^T blocks     [l, m]
    rowmax on DVE, tanh/exp on ACT, H cast on ACT, F^T/linear bf16 on PE.

Sharding: pure data parallel, 64 batches per core across 8 cores. The
embedding lookup (a pure data relayout) happens on host: each core gets its
tokens' embedding rows in tile layout (bf16) plus their transpose (fp8),
loaded with large linear DMAs on the two HWDGE rings.
"""

import os
import sys

for _p in ("/opt/trn_rl_repo", "/root/.axon_site/_ro/trn_rl_repo"):
    if os.path.isdir(_p) and _p not in sys.path:
        sys.path.insert(0, _p)

import ml_dtypes
import numpy as np

B, L, D, V, O = 512, 256, 256, 100000, 1000
NCORES = 8
BPC = B // NCORES  # 64 batches per core
NB = 16            # batches per chunk
NCH = BPC // NB    # 4 chunks
NPAIR = NB // 2    # 8 batch-pairs per chunk
OPAD = 1024        # output dim padded to 8*128

_CACHE: dict = {}


def _build_bass(reps=1, skip=()):
    from contextlib import nullcontext

    import concourse.bass as bass
    import concourse.tile as tile
    from concourse import bacc, mybir

    nc = bacc.Bacc("TRN2", target_bir_lowering=False, debug=False, num_devices=NCORES)
    bf = mybir.dt.bfloat16
    f32 = mybir.dt.float32
    f8 = mybir.dt.float8e4

    eg = nc.dram_tensor("eg", [128, NCH, 2 * NB, D], bf, kind="ExternalInput")
    etg = nc.dram_tensor(
        "etg", [128, NCH, NPAIR, 2, 2 * L], f8, kind="ExternalInput"
    )
    wbt = nc.dram_tensor("wbt", [128, 2, D], f8, kind="ExternalInput")
    lwt = nc.dram_tensor("lwt", [128, 2, OPAD], bf, kind="ExternalInput")
    lb = nc.dram_tensor("lb", [1, OPAD], f32, kind="ExternalInput")
    onc = nc.dram_tensor("onc", [128, 1], f32, kind="ExternalInput")
    onr = nc.dram_tensor("onr", [1, 128], f32, kind="ExternalInput")
    # transposed output layout [o, b]; host transposes back (free)
    out = nc.dram_tensor("out", [OPAD, BPC], f32, kind="ExternalOutput")

    with tile.TileContext(nc) as tc:
        with (
            tc.tile_pool(name="const", bufs=1) as constp,
            tc.tile_pool(name="ftp", bufs=1, space="PSUM") as ftp,
            tc.tile_pool(name="small", bufs=2) as smallp,
        ):
            # sync ring: wbt + eg chunks; scalar ring: etg chunks; the
            # end-game consts ride the otherwise idle Pool SWDGE ring.
            wbt_sb = constp.tile([128, 2, D], f8)
            nc.sync.dma_start(wbt_sb[:], wbt[:])
            lwt_sb = constp.tile([128, 2, OPAD], bf)
            nc.gpsimd.dma_start(lwt_sb[:], lwt[:])
            lb_sb = constp.tile([1, OPAD], f32)
            nc.gpsimd.dma_start(lb_sb[:], lb[:])
            onc_sb = constp.tile([128, 1], f32)
            nc.gpsimd.dma_start(onc_sb[:], onc[:])
            onr_sb = constp.tile([1, 128], f32)
            nc.gpsimd.dma_start(onr_sb[:], onr[:])

            rep_cm = (
                tc.For_i(0, reps, 1, hint_engines=tuple(nc.engines.keys()))
                if reps > 1
                else nullcontext()
            )
            with rep_cm:
                # F^T accumulator: [d % 128, d // 128, batch], unnormalized
                ft_ps = ftp.tile([128, 2, BPC], f32)
                # unnormalized softmax weights for all batches
                w_all = smallp.tile([128, 2, BPC], f32, tag="wall")
                _kernel_body(
                    nc, tc, mybir, bf, f32, f8,
                    wbt_sb, lwt_sb, lb_sb, onc_sb, onr_sb,
                    ft_ps, w_all, eg, etg, out, smallp, skip,
                )

    nc.compile()
    return nc


def _kernel_body(
    nc, tc, mybir, bf, f32, f8,
    wbt_sb, lwt_sb, lb_sb, onc_sb, onr_sb,
    ft_ps, w_all, eg, etg, out, smallp, skip=(),
):
    Copy = mybir.ActivationFunctionType.Copy
    Tanh = mybir.ActivationFunctionType.Tanh
    Exp = mybir.ActivationFunctionType.Exp
    AX = mybir.AxisListType.X
    DR = mybir.MatmulPerfMode.DoubleRow

    with (
        tc.tile_pool(name="eplain", bufs=2) as ep,
        tc.tile_pool(name="etsb", bufs=2) as etsbp,
        tc.tile_pool(name="hps", bufs=3, space="PSUM") as hpsp,
        tc.tile_pool(name="hsb", bufs=3) as hsbp,
        tc.tile_pool(name="mps", bufs=2, space="PSUM") as mpsp,
        tc.tile_pool(name="rps", bufs=1, space="PSUM") as rpsp,
        tc.tile_pool(name="ops", bufs=1, space="PSUM") as opsp,
    ):
        zp = rpsp.tile([1, BPC], f32, tag="zp")

        def emit_ft(Eprev, wnprev, w32prev, cprev, btlo=0, bthi=NB):
            # F^T[:, k, col] += E_block^T @ wn  (unnormalized weighted sum)
            for bt in range(btlo, bthi):
                col = cprev * NB + bt
                for k in range(2):
                    for h in range(2):
                        nc.tensor.matmul(
                            out=ft_ps[:, k:k + 1, col:col + 1],
                            lhsT=Eprev[:, 2 * bt + h:2 * bt + h + 1, k * 128:(k + 1) * 128],
                            rhs=wnprev[:, h:h + 1, bt:bt + 1],
                            start=(h == 0),
                            stop=(h == 1),
                        )
            # Z partial for the chunk: zp[0, col] = sum_l w
            for h in range(2):
                nc.tensor.matmul(
                    out=zp[:, cprev * NB + btlo:cprev * NB + bthi],
                    lhsT=onc_sb[:],
                    rhs=w32prev[:, h:h + 1, btlo:bthi],
                    start=(h == 0),
                    stop=(h == 1),
                )

        def emit_out(cprev):
            # per-chunk endgame: rz, broadcast, normalize F^T, then the
            # linear TRANSPOSED (out^T[o, b]: o on partitions) so the PSUM
            # drain is one [128, 8, NB] copy instead of a [NB, 1024] one
            sl = slice(cprev * NB, (cprev + 1) * NB)
            rz = smallp.tile([1, NB], f32, tag="rz")
            nc.vector.reciprocal(rz[:], zp[:, sl])
            r2s = smallp.tile([128, NB], f32, tag="r2s")
            nc.gpsimd.partition_broadcast(r2s[:], rz[:])
            fts = smallp.tile([128, 2, NB], bf, tag="fts")
            for k in range(2):
                nc.vector.tensor_mul(fts[:, k:k + 1, :], ft_ps[:, k:k + 1, sl], r2s[:])
            op = opsp.tile([128, 8, NB], f32, tag="op")
            for g in range(8):
                for k in range(2):
                    nc.tensor.matmul(
                        out=op[:, g:g + 1, :],
                        lhsT=lwt_sb[:, k:k + 1, g * 128:(g + 1) * 128],
                        rhs=fts[:, k:k + 1, :],
                        start=(k == 0), stop=False, skip_group_check=True,
                    )
                nc.tensor.matmul(
                    out=op[:, g:g + 1, :],
                    lhsT=lb_sb[:, g * 128:(g + 1) * 128],
                    rhs=onr_sb[:, :NB],
                    start=False, stop=True, skip_group_check=True,
                )
            osb = smallp.tile([128, 8, NB], f32, tag="osb")
            nc.scalar.copy(osb[:], op[:])
            nc.sync.dma_start(out.rearrange("(g p) b -> p g b", p=128)[:, :, sl], osb[:])

        def emit_tail(c, btlo, bthi):
            # w = exp(tanh(rm / 256)), kept unnormalized
            rm = rm_tiles[c]
            n = bthi - btlo
            t32 = smallp.tile([128, 2, n], f32, tag="t32")
            nc.scalar.activation(
                t32[:], rm[:, :, btlo:bthi], Tanh, scale=1.0 / 256.0
            )
            w32 = w_all[:, :, c * NB:(c + 1) * NB]
            nc.scalar.activation(w32[:, :, btlo:bthi], t32[:], Exp)
            wn = smallp.tile([128, 2, NB], bf, tag="wn")
            nc.gpsimd.tensor_copy(wn[:, :, btlo:bthi], w32[:, :, btlo:bthi])
            return wn, w32

        def dma_etc(c, quarters=False):
            etc = etsbp.tile([128, NPAIR, 2, 2 * L], f8, tag="etc")
            if quarters:
                # chunk 0 on the scalar ring, in quarters: first H starts early
                for q in range(4):
                    nc.scalar.dma_start(
                        etc[:, 2 * q:2 * q + 2, :, :],
                        etg[:, c, 2 * q:2 * q + 2, :, :],
                    )
            else:
                # prefetched a chunk ahead on the otherwise idle Pool ring
                nc.gpsimd.dma_start(etc[:], etg[:, c, :, :, :])
            return etc

        prev = None
        etc = None
        rm_tiles = {}
        for c in range(NCH):
            # E[l%128, 2*bt + l//128, d] bf16 (for the weighted sum) and
            # ET[d%128, chunk-pair, d//128, (b0 l)|(b1 l)] fp8 x16 (for H/M).
            E = ep.tile([128, 2 * NB, D], bf, tag="E")
            if "dma" not in skip:
                nc.sync.dma_start(E[:], eg[:, c, :, :])
                if c == 0:
                    etc = dma_etc(0, quarters=True)
            else:
                nc.vector.memset(E[:, 0, 0:8], 0.125)
                if c == 0:
                    etc = etsbp.tile([128, NPAIR, 2, 2 * L], f8, tag="etc")
                    nc.vector.memset(etc[:, 0, 0, 0:8], 0.125)

            if "compute" in skip:
                # keep the loads live with a minimal consumer
                sc = smallp.tile([128, 64], bf, tag="sc")
                nc.vector.tensor_copy(sc[:], E[:, 0, 0:64])
                sc2 = smallp.tile([128, 64], bf, tag="sc2")
                nc.vector.tensor_copy(sc2[:], etc[:, 0, 0, 0:64])
                if c > 0 and "dma" not in skip:
                    etc = dma_etc(c)
                osb = smallp.tile([128, 8, NB], f32, tag="osb")
                nc.vector.memset(osb[:, 0, 0:8], 0.125)
                sl = slice(c * NB, (c + 1) * NB)
                nc.sync.dma_start(
                    out.rearrange("(g p) b -> p g b", p=128)[:, :, sl], osb[:]
                )
                continue

            rm = smallp.tile([128, 2, NB], f32, tag="rm")
            rm_tiles[c] = rm
            pend = None  # (ets, hs, p) whose M is not yet emitted
            etc_next = None
            for p in range(NPAIR):
                ets = etc[:, p, :, :]
                # H = W_b @ E^T both batches, fp8 DoubleRow: K=256 per instr;
                # cast 256*H -> 16*H in fp8 (scale 2^-4, exact)
                hs = hsbp.tile([128, 2, 2 * L], f8, tag="hs")
                for t in range(2):
                    hp = hpsp.tile([128, 1, 2 * L], f32, tag="hp")
                    nc.tensor.matmul(
                        out=hp[:, 0:1, :],
                        lhsT=wbt_sb[:, :, t * 128:(t + 1) * 128],
                        rhs=ets[:],
                        start=True,
                        stop=True,
                        perf_mode=DR,
                    )
                    nc.scalar.activation(hs[:, t:t + 1, :], hp[:], Copy, scale=0.0625)

                def emit_m(ets, hs, p):
                    # M = E @ H per batch, fp8 DoubleRow; rowmax over m
                    for j in range(2):
                        mp = mpsp.tile([128, 2, L], f32, tag="mp")
                        for h in range(2):
                            lo = j * L + h * 128
                            nc.tensor.matmul(
                                out=mp[:, h:h + 1, :],
                                lhsT=ets[:, :, lo:lo + 128],
                                rhs=hs[:, :, j * L:(j + 1) * L],
                                start=True,
                                stop=True,
                                perf_mode=DR,
                            )
                        nc.vector.reduce_max(
                            out=rm[:, :, 2 * p + j:2 * p + j + 1], in_=mp[:], axis=AX
                        )

                if pend is not None:
                    emit_m(*pend)
                pend = (ets, hs, p)
                if p == 0:
                    if prev is not None:
                        # previous chunk's weighted sum + Z: PE filler that
                        # also covers this chunk's first H->cast latency
                        emit_ft(*prev)
                    if c + 1 < NCH and "dma" not in skip:
                        etc_next = dma_etc(c + 1)
                if p == 2 and prev is not None:
                    emit_out(prev[3])
                if c == NCH - 1 and p == 5:
                    # last chunk: first-half tail early (rm cols 0..7 are
                    # final once emit_m(p=4) above has been emitted)
                    wha = emit_tail(c, 0, NB // 2)
                    emit_ft(E, wha[0], wha[1], c, 0, NB // 2)
            emit_m(*pend)

            if c == NCH - 1:
                whb = emit_tail(c, NB // 2, NB)
                prev = (E, whb[0], whb[1], c)
            else:
                wh = emit_tail(c, 0, NB)
                prev = (E, wh[0], wh[1], c)
            etc = etc_next

        if "compute" not in skip:
            emit_ft(*prev, NB // 2, NB)
            emit_out(prev[3])


def _get_nc(reps=1, skip=()):
    key = ("nc", reps, tuple(skip))
    if key not in _CACHE:
        _CACHE[key] = _build_bass(reps=reps, skip=skip)
    return _CACHE[key]


def _prep_in_maps(input_sentence, emb_weight, W_b, lin_w, lin_b):
    bfl = ml_dtypes.bfloat16
    f8l = ml_dtypes.float8_e4m3
    tokens = np.asarray(input_sentence).astype(np.int64)
    emb_f = np.ascontiguousarray(np.asarray(emb_weight, dtype=np.float32))
    emb_bf = emb_f.astype(bfl)

    # replicated weights; W_b scaled by 16 into fp8 (values ~1, no denormals)
    wbt_f = np.asarray(W_b, dtype=np.float32).T.reshape(2, 128, D).transpose(1, 0, 2)
    wbt8 = np.ascontiguousarray(16.0 * wbt_f).astype(f8l)
    lwt_pad = np.zeros((D, OPAD), dtype=np.float32)
    lwt_pad[:, :O] = np.asarray(lin_w, dtype=np.float32).T
    lwt = np.ascontiguousarray(lwt_pad.reshape(2, 128, OPAD).transpose(1, 0, 2)).astype(bfl)
    lb_pad = np.zeros((1, OPAD), dtype=np.float32)
    lb_pad[0, :O] = np.asarray(lin_b, dtype=np.float32)
    onc = np.ones((128, 1), dtype=np.float32)
    onr = np.ones((1, 128), dtype=np.float32)

    in_maps = []
    for ci in range(NCORES):
        shard = tokens[ci * BPC:(ci + 1) * BPC]  # [64, 256]
        # eg[p, c, 2*bt+h, :] = emb[tok[c*NB+bt, h*128+p]]
        idx = shard.reshape(NCH, NB, 2, 128).transpose(3, 0, 1, 2).reshape(
            128, NCH, 2 * NB
        )
        eg = emb_bf[idx]  # [128, NCH, 2*NB, D]
        # etg[dp, c, p, k, j*L + l] = fp8(16 * E_b[l, k*128+dp]), b=(c, p, j)
        Eall = emb_f[shard]  # [BPC, L, D] f32
        et = (16.0 * Eall).transpose(0, 2, 1)  # [b, d, l]
        etg = np.ascontiguousarray(
            et.reshape(NCH, NPAIR, 2, 2, 128, L)
            .transpose(4, 0, 1, 3, 2, 5)
            .reshape(128, NCH, NPAIR, 2, 2 * L)
        ).astype(f8l)
        in_maps.append(
            {
                "eg": np.ascontiguousarray(eg),
                "etg": etg,
                "wbt": wbt8,
                "lwt": lwt,
                "lb": lb_pad,
                "onc": onc,
                "onr": onr,
            }
        )
    return in_maps


def _run(in_maps, trace=False):
    from concourse.bass_utils import run_bass_kernel_spmd

    return run_bass_kernel_spmd(_get_nc(), in_maps, list(range(NCORES)), trace=trace)


def kernel(input_sentence, emb_weight, W_b, lin_w, lin_b):
    in_maps = _prep_in_maps(input_sentence, emb_weight, W_b, lin_w, lin_b)
    res = _run(in_maps)
    full = np.concatenate([np.asarray(r["out"]).T for r in res.results], axis=0)
    return np.ascontiguousarray(full[:, :O]).astype(np.float32)
